# revision 28
# baseline (speedup 1.0000x reference)
"""ANI-style element-MLP (MoE routing) kernel for 8 TRN2 NeuronCores.

Strategy:
  - Host: bucket atoms by element (expert). Only ~4/9 of atoms match any
    expert; the rest contribute 0.  Each expert bucket is padded to a fixed
    capacity, split in half, and each half is assigned to one core
    (cores 2e, 2e+1 own expert e).  Per-core inputs are the gathered,
    transposed representation rows [D, S] plus that expert's weights laid
    out in SBUF-ready [128, ...] chunk order.
  - Device: 3-layer MLP as tiled matmuls (features on partitions so biases
    are per-partition ACT bias), softplus on the scalar engine.  The
    softplus -log(2) shift is folded into the next layer's bias on host.
    Output is the per-slot scalar energy [1, S] per core.
  - Host: scatter-add real slots' energies into the per-molecule output [B].

Self-contained: hardcodes problem shapes B=32, N=512, D=384, E=4, H=256.
"""

import os

import ml_dtypes
import numpy as np

import concourse.bass as bass  # noqa: F401  (bass types referenced via bacc/mybir)
import concourse.env as _cenv
import concourse.mybir as mybir
import concourse.bass_utils as _bass_utils
from concourse import bacc
from concourse.bass_utils import run_bass_kernel_spmd
from concourse.hw_specs import get_activation_tables
from concourse.tile import TileContext

# The NEFF epilogue walrus emits restores every semaphore up to the highest
# one used (~115ns each, serialized per engine) — with the default kernel-sem
# base of 150 that's a ~6us fixed tail.  78 is the documented static minimum
# for walrus's own needs; compacting the kernel sems down shrinks the restore
# loop.  Both the bass-side base and walrus's --max-sem-num must agree.
_WALRUS_MAX_SEM = 78
_cenv.get_walrus_max_sem_num = lambda: _WALRUS_MAX_SEM
bass.get_walrus_max_sem_num = lambda: _WALRUS_MAX_SEM

_orig_bvo = _bass_utils.bir_verify_and_optimise


def _bvo_with_sem_cap(*args, **kwargs):
    orig_run = _bass_utils.run_command

    def run_patched(cmd, *a, **kw):
        if isinstance(cmd, list) and cmd and "walrus_driver" in str(cmd[0]):
            cmd = list(cmd) + [f"--max-sem-num={_WALRUS_MAX_SEM}"]
        return orig_run(cmd, *a, **kw)

    _bass_utils.run_command = run_patched
    try:
        return _orig_bvo(*args, **kwargs)
    finally:
        _bass_utils.run_command = orig_run


_bass_utils.bir_verify_and_optimise = _bvo_with_sem_cap


class _OneActSetBacc(bacc.Bacc):
    """All our ACT functions (Exp, Ln, Identity) live in the
    natural_log_exp_and_others table set, but the stock table-load pass
    assigns each function its first matching set, thrashing ~1.5us table
    loads between sets on every layer.  Force every load to the one set
    that covers all three and drop the now-redundant reloads."""

    _ACT_SET = "natural_log_exp_and_others"

    def insert_act_table_loads(self):
        super().insert_act_table_loads()
        names = list(get_activation_tables(self.m.arch))
        target = names.index(self._ACT_SET)
        for blk in self.main_func.blocks:
            seen_engines = set()
            to_remove = []
            for inst in blk.instructions:
                if isinstance(inst, mybir.InstLoadActFuncSet):
                    if inst.engine in seen_engines and not (inst.has_wait() or inst.has_update()):
                        to_remove.append(inst)
                    else:
                        inst.act_func_set_id = target
                        seen_engines.add(inst.engine)
            for inst in to_remove:
                blk.instructions.remove(inst)

LOG2 = np.float32(np.log(2.0))
B, N, D = 32, 512, 384
E = 4
H1 = H2 = 256
N_CORES = 8
NT = 512  # moving-operand (slot) tile for matmuls; one PSUM bank at f32

F32 = mybir.dt.float32

# Set by test harnesses: PROFILE=True makes kernel() run with NTFF tracing and
# store the profiled NEFF exec time (ns) in LAST_EXEC_NS.
PROFILE = False
TRACE_CORES = [0]
LAST_EXEC_NS = None

_CACHE: dict = {}


BF16 = mybir.dt.bfloat16


def _build(S: int):
    """Raw-Bass per-core graph for S slots (one expert per core).

    Engine plan (explicit semaphores, no Tile):
      sync   : x DMAs in, final out DMA
      scalar : weight/bias DMAs (2nd HWDGE queue), all Exp/Ln activations
      tensor : all matmuls (z1/z2 per slot-chunk + the W3 row, PSUM-aliased)
      vector : +b3 epilogue copy PSUM->SBUF out
    """
    from contextlib import ExitStack

    nc = _OneActSetBacc(None, target_bir_lowering=False)

    x_ext = nc.declare_dram_parameter("x", [128, 3 * S], BF16, isOutput=False)
    wt_ext = nc.declare_dram_parameter("wt", [128, 1282], BF16, isOutput=False)
    bias_ext = nc.declare_dram_parameter("bias", [128, 5], F32, isOutput=False)
    out_ext = nc.declare_dram_parameter("out", [1, S], F32, isOutput=True)

    EXP = mybir.ActivationFunctionType.Exp
    LN = mybir.ActivationFunctionType.Ln
    ID = mybir.ActivationFunctionType.Identity

    TCH = S // NT  # slot chunks (2 for S=1024)
    assert TCH == 2, "sem schedule below is written for 2 slot chunks"

    with ExitStack() as ctx:
        xt = ctx.enter_context(nc.sbuf_tensor([128, 3 * S], BF16))
        wt = ctx.enter_context(nc.sbuf_tensor([128, 1282], BF16))
        bias = ctx.enter_context(nc.sbuf_tensor([128, 5], F32))
        scratch = ctx.enter_context(nc.sbuf_tensor([1, 16], F32))
        out_sb = ctx.enter_context(nc.sbuf_tensor([1, S], F32))
        t1 = [ctx.enter_context(nc.sbuf_tensor(f"t1_{t}", [128, 2 * NT], F32)) for t in range(TCH)]
        a1 = [ctx.enter_context(nc.sbuf_tensor(f"a1_{t}", [128, 2 * NT], BF16)) for t in range(TCH)]
        t2 = [ctx.enter_context(nc.sbuf_tensor(f"t2_{t}", [128, 2 * NT], F32)) for t in range(TCH)]
        a2 = [ctx.enter_context(nc.sbuf_tensor(f"a2_{t}", [128, 2 * NT], BF16)) for t in range(TCH)]
        z1 = [ctx.enter_context(nc.psum_tensor(f"z1_{t}", [128, 2 * NT], F32)) for t in range(TCH)]
        z2 = [ctx.enter_context(nc.psum_tensor(f"z2_{t}", [128, 2 * NT], F32)) for t in range(TCH)]
        sem_x = [[ctx.enter_context(nc.semaphore(f"sem_x{t}{d}")) for d in range(3)] for t in range(TCH)]
        sem_w = ctx.enter_context(nc.semaphore("sem_w"))
        sem_b = ctx.enter_context(nc.semaphore("sem_b"))
        sem_o = ctx.enter_context(nc.semaphore("sem_o"))
        sem_mm = ctx.enter_context(nc.semaphore("sem_mm"))
        sem_act = ctx.enter_context(nc.semaphore("sem_act"))
        sem_v = ctx.enter_context(nc.semaphore("sem_v"))
        block = ctx.enter_context(nc.Block())

        # the W3 energy row reuses z2[t]'s first bank, partition 0 (its
        # matmuls run only after the Exps have drained z2[t])
        er = [z2[t][0:1, 0:NT] for t in range(TCH)]

        def w1s(d, h):
            return wt[:, (d * 2 + h) * 128 : (d * 2 + h + 1) * 128]

        def w2s(h, k):
            return wt[:, 768 + (h * 2 + k) * 128 : 768 + (h * 2 + k + 1) * 128]

        def w3s(k):
            return wt[:, 1280 + k : 1281 + k]

        @block.sync
        def _(sync):
            # host supplies x pre-laid-out as [128, t*(3*NT) + d*NT + s].
            # x is split across both HWDGE rings (SP + ACT) so the two rings
            # pull in parallel; d0+d1 of each chunk on SP, d2 on ACT.
            for t in range(TCH):
                for d in range(2):
                    c = (t * 3 + d) * NT
                    sync.dma_start(xt[:, c : c + NT], x_ext[:, c : c + NT]).then_inc(sem_x[t][d], 16)
            sync.dma_start(bias[:], bias_ext[:]).then_inc(sem_b, 16)
            sync.wait_ge(sem_v, 1)
            sync.dma_start(out_ext[:, 0:NT], out_sb[:, 0:NT]).then_inc(sem_o, 16)
            sync.wait_ge(sem_v, 2)
            sync.dma_start(out_ext[:, NT : 2 * NT], out_sb[:, NT : 2 * NT]).then_inc(sem_o, 16)
            sync.wait_ge(sem_o, 32)

        @block.scalar
        def _(scalar):
            scalar.dma_start(wt[:], wt_ext[:]).then_inc(sem_w, 16)
            for t in range(TCH):
                c = (t * 3 + 2) * NT
                scalar.dma_start(xt[:, c : c + NT], x_ext[:, c : c + NT]).then_inc(sem_x[t][2], 16)
            # memzero lowers to an ACTIVATE, anchoring the ACT table load
            # before any cross-engine waits
            scalar.memzero(scratch[:])
            scalar.wait_ge(sem_b, 16)
            # PE sem_mm cumulative: z1(0)=1, z1(1)=2, z2(0)=3, z2(1)=4,
            # er(0)=5, er(1)=6.
            # sem_act: li 0..2 -> 3 incs each (exp,exp,ln); li 3 -> 4 incs
            # (exp,exp,ln half,ln half) so er(1) can start on the first half.
            for li, zz, tt, aa, bcol in (
                (0, z1, t1, a1, 0),
                (1, z1, t1, a1, 0),
                (2, z2, t2, a2, 2),
                (3, z2, t2, a2, 2),
            ):
                t = li % 2
                scalar.wait_ge(sem_mm, li + 1)
                for h in range(2):
                    scalar.activation(
                        tt[t][:, h * NT : (h + 1) * NT],
                        zz[t][:, h * NT : (h + 1) * NT],
                        EXP,
                        bias=bias[:, bcol + h : bcol + h + 1],
                    ).then_inc(sem_act, 1)
                scalar.wait_ge(sem_act, 3 * li + 2)  # ACT pipeline RAW: exp fully written
                if li < 3:
                    scalar.activation(aa[t][:], tt[t][:], LN, bias=1.0).then_inc(sem_act, 1)
                else:
                    for k in range(2):
                        scalar.activation(
                            aa[t][:, k * NT : (k + 1) * NT],
                            tt[t][:, k * NT : (k + 1) * NT],
                            LN,
                            bias=1.0,
                        ).then_inc(sem_act, 1)

        @block.tensor
        def _(tensor):
            def l1(t):
                # d-outer so matmuls start as soon as each x d-chunk lands
                for d in range(3):
                    tensor.wait_ge(sem_x[t][d], 16)
                    for h in range(2):
                        mm = tensor.matmul(
                            z1[t][:, h * NT : (h + 1) * NT],
                            w1s(d, h),
                            xt[:, (t * 3 + d) * NT : (t * 3 + d + 1) * NT],
                            start=(d == 0),
                            stop=(d == 2),
                            skip_group_check=True,
                        )
                mm.then_inc(sem_mm, 1)

            def l2(t):
                for k in range(2):
                    for h in range(2):
                        mm = tensor.matmul(
                            z2[t][:, k * NT : (k + 1) * NT],
                            w2s(h, k),
                            a1[t][:, h * NT : (h + 1) * NT],
                            start=(h == 0),
                            stop=(h == 1),
                        )
                mm.then_inc(sem_mm, 1)

            def l3(t, act_waits):
                for k in range(2):
                    tensor.wait_ge(sem_act, act_waits[k])
                    mm = tensor.matmul(
                        er[t],
                        w3s(k),
                        a2[t][:, k * NT : (k + 1) * NT],
                        start=(k == 0),
                        stop=(k == 1),
                        skip_group_check=True,
                    )
                mm.then_inc(sem_mm, 1)

            tensor.wait_ge(sem_w, 16)
            l1(0)  # -> 1
            l1(1)  # -> 2
            tensor.wait_ge(sem_act, 3)
            l2(0)  # -> 3
            tensor.wait_ge(sem_act, 6)
            l2(1)  # -> 4
            l3(0, (9, 9))  # -> 5
            l3(1, (12, 13))  # -> 6

        @block.vector
        def _(vector):
            for t in range(TCH):
                vector.wait_ge(sem_mm, 5 + t)
                vector.tensor_scalar_add(
                    out_sb[:, t * NT : (t + 1) * NT], er[t], bias[0:1, 4:5]
                ).then_inc(sem_v, 1)

    nc.finalize()
    return nc


def kernel(representation, atomic_numbers, elements, W1, b1, W2, b2, W3, b3):
    global LAST_EXEC_NS
    rep = np.asarray(representation, dtype=np.float32)
    an = np.asarray(atomic_numbers).astype(np.int64)
    el = np.asarray(elements).astype(np.int64)
    W1 = np.asarray(W1, dtype=np.float32)
    b1 = np.asarray(b1, dtype=np.float32)
    W2 = np.asarray(W2, dtype=np.float32)
    b2 = np.asarray(b2, dtype=np.float32)
    W3 = np.asarray(W3, dtype=np.float32)
    b3 = np.asarray(b3, dtype=np.float32)

    Bsz, Nn, Dd = rep.shape
    flat = rep.reshape(-1, Dd)
    anf = an.reshape(-1)

    idxs = [np.nonzero(anf == el[e])[0] for e in range(E)]
    counts = [len(ix) for ix in idxs]

    # slots per core; expert capacity = 2*S (two cores per expert)
    S = 1024
    while max(counts) > 2 * S:
        S *= 2

    # fold the shifted-softplus -log(2) into downstream biases
    b2_eff = b2 - LOG2 * W2.sum(axis=1)  # [E, H2]
    b3_eff = b3 - LOG2 * W3.sum(axis=1)  # [E]

    if S not in _CACHE:
        _CACHE[S] = _build(S)
    nc = _CACHE[S]

    in_maps = []
    for c in range(N_CORES):
        e, half = divmod(c, 2)
        ix = idxs[e]
        lo = half * S
        hi = min(len(ix), lo + S)
        bf16 = ml_dtypes.bfloat16
        xs = np.zeros((S, Dd), np.float32)
        if hi > lo:
            xs[: hi - lo] = flat[ix[lo:hi]]
        wt = np.zeros((128, 1282), np.float32)
        wt[:, 0:768] = W1[e].reshape(3, 128, 2, 128).transpose(1, 0, 2, 3).reshape(128, 768)
        wt[:, 768:1280] = W2[e].reshape(2, 128, 2, 128).transpose(1, 0, 2, 3).reshape(128, 512)
        wt[:, 1280:1282] = W3[e].reshape(2, 128).T
        bias = np.zeros((128, 5), np.float32)
        bias[:, 0:2] = b1[e].reshape(2, 128).T
        bias[:, 2:4] = b2_eff[e].reshape(2, 128).T
        bias[0, 4] = b3_eff[e]
        in_maps.append(
            {
                "x": np.ascontiguousarray(
                    xs.T.reshape(3, 128, S // NT, NT).transpose(1, 2, 0, 3).reshape(128, 3 * S)
                ).astype(bf16),
                "wt": wt.astype(bf16),
                "bias": bias,
            }
        )

    kwargs = {}
    if PROFILE:
        kwargs = dict(trace=True, trace_cores=list(TRACE_CORES))
    res = run_bass_kernel_spmd(nc, in_maps, core_ids=list(range(N_CORES)), **kwargs)
    LAST_EXEC_NS = res.exec_time_ns

    energies = np.zeros(Bsz, np.float64)
    for c in range(N_CORES):
        e, half = divmod(c, 2)
        ix = idxs[e]
        lo = half * S
        hi = min(len(ix), lo + S)
        if hi <= lo:
            continue
        evals = np.asarray(res.results[c]["out"]).reshape(-1)[: hi - lo]
        np.add.at(energies, ix[lo:hi] // Nn, evals.astype(np.float64))
    return energies.astype(np.float32)


# revision 29
# speedup vs baseline: 1.0325x; 1.0325x over previous
"""ANI-style element-MLP (MoE routing) kernel for 8 TRN2 NeuronCores.

Strategy:
  - Host: bucket atoms by element (expert). Only ~4/9 of atoms match any
    expert; the rest contribute 0.  Each expert bucket is padded to a fixed
    capacity, split in half, and each half is assigned to one core
    (cores 2e, 2e+1 own expert e).  Per-core inputs are the gathered,
    transposed representation rows [D, S] plus that expert's weights laid
    out in SBUF-ready [128, ...] chunk order.
  - Device: 3-layer MLP as tiled matmuls (features on partitions so biases
    are per-partition ACT bias), softplus on the scalar engine.  The
    softplus -log(2) shift is folded into the next layer's bias on host.
    Output is the per-slot scalar energy [1, S] per core.
  - Host: scatter-add real slots' energies into the per-molecule output [B].

Self-contained: hardcodes problem shapes B=32, N=512, D=384, E=4, H=256.
"""

import os

import ml_dtypes
import numpy as np

import concourse.bass as bass  # noqa: F401  (bass types referenced via bacc/mybir)
import concourse.env as _cenv
import concourse.mybir as mybir
import concourse.bass_utils as _bass_utils
from concourse import bacc
from concourse.bass_utils import run_bass_kernel_spmd
from concourse.hw_specs import get_activation_tables
from concourse.tile import TileContext

# The NEFF epilogue walrus emits restores every semaphore up to the highest
# one used (~115ns each, serialized per engine) — with the default kernel-sem
# base of 150 that's a ~6us fixed tail.  78 is the documented static minimum
# for walrus's own needs; compacting the kernel sems down shrinks the restore
# loop.  Both the bass-side base and walrus's --max-sem-num must agree.
_WALRUS_MAX_SEM = 78
_cenv.get_walrus_max_sem_num = lambda: _WALRUS_MAX_SEM
bass.get_walrus_max_sem_num = lambda: _WALRUS_MAX_SEM

_orig_bvo = _bass_utils.bir_verify_and_optimise


def _bvo_with_sem_cap(*args, **kwargs):
    orig_run = _bass_utils.run_command

    def run_patched(cmd, *a, **kw):
        if isinstance(cmd, list) and cmd and "walrus_driver" in str(cmd[0]):
            cmd = list(cmd) + [f"--max-sem-num={_WALRUS_MAX_SEM}"]
        return orig_run(cmd, *a, **kw)

    _bass_utils.run_command = run_patched
    try:
        return _orig_bvo(*args, **kwargs)
    finally:
        _bass_utils.run_command = orig_run


_bass_utils.bir_verify_and_optimise = _bvo_with_sem_cap


class _OneActSetBacc(bacc.Bacc):
    """All our ACT functions (Exp, Ln, Identity) live in the
    natural_log_exp_and_others table set, but the stock table-load pass
    assigns each function its first matching set, thrashing ~1.5us table
    loads between sets on every layer.  Force every load to the one set
    that covers all three and drop the now-redundant reloads."""

    _ACT_SET = "natural_log_exp_and_others"

    def insert_act_table_loads(self):
        super().insert_act_table_loads()
        names = list(get_activation_tables(self.m.arch))
        target = names.index(self._ACT_SET)
        for blk in self.main_func.blocks:
            seen_engines = set()
            to_remove = []
            for inst in blk.instructions:
                if isinstance(inst, mybir.InstLoadActFuncSet):
                    if inst.engine in seen_engines and not (inst.has_wait() or inst.has_update()):
                        to_remove.append(inst)
                    else:
                        inst.act_func_set_id = target
                        seen_engines.add(inst.engine)
            for inst in to_remove:
                blk.instructions.remove(inst)

LOG2 = np.float32(np.log(2.0))
B, N, D = 32, 512, 384
E = 4
H1 = H2 = 256
N_CORES = 8
NT = 512  # moving-operand (slot) tile for matmuls; one PSUM bank at f32

F32 = mybir.dt.float32

# Set by test harnesses: PROFILE=True makes kernel() run with NTFF tracing and
# store the profiled NEFF exec time (ns) in LAST_EXEC_NS.
PROFILE = False
TRACE_CORES = [0]
LAST_EXEC_NS = None

_CACHE: dict = {}


BF16 = mybir.dt.bfloat16


def _build(S: int):
    """Raw-Bass per-core graph for S slots (one expert per core).

    Engine plan (explicit semaphores, no Tile):
      sync   : x DMAs in, final out DMA
      scalar : weight/bias DMAs (2nd HWDGE queue), all Exp/Ln activations
      tensor : all matmuls (z1/z2 per slot-chunk + the W3 row, PSUM-aliased)
      vector : +b3 epilogue copy PSUM->SBUF out
    """
    from contextlib import ExitStack

    nc = _OneActSetBacc(None, target_bir_lowering=False)

    x_ext = nc.declare_dram_parameter("x", [128, 3 * S], BF16, isOutput=False)
    wt_ext = nc.declare_dram_parameter("wt", [128, 1282], BF16, isOutput=False)
    bias_ext = nc.declare_dram_parameter("bias", [128, 5], F32, isOutput=False)
    out_ext = nc.declare_dram_parameter("out", [1, S], F32, isOutput=True)

    EXP = mybir.ActivationFunctionType.Exp
    LN = mybir.ActivationFunctionType.Ln
    ID = mybir.ActivationFunctionType.Identity

    TCH = S // NT  # slot chunks (2 for S=1024)
    assert TCH == 2, "sem schedule below is written for 2 slot chunks"

    with ExitStack() as ctx:
        xt = ctx.enter_context(nc.sbuf_tensor([128, 3 * S], BF16))
        wt = ctx.enter_context(nc.sbuf_tensor([128, 1282], BF16))
        bias = ctx.enter_context(nc.sbuf_tensor([128, 5], F32))
        scratch = ctx.enter_context(nc.sbuf_tensor([1, 16], F32))
        out_sb = ctx.enter_context(nc.sbuf_tensor([1, S], F32))
        t1 = [ctx.enter_context(nc.sbuf_tensor(f"t1_{t}", [128, 2 * NT], F32)) for t in range(TCH)]
        a1 = [ctx.enter_context(nc.sbuf_tensor(f"a1_{t}", [128, 2 * NT], BF16)) for t in range(TCH)]
        t2 = [ctx.enter_context(nc.sbuf_tensor(f"t2_{t}", [128, 2 * NT], F32)) for t in range(TCH)]
        a2 = [ctx.enter_context(nc.sbuf_tensor(f"a2_{t}", [128, 2 * NT], BF16)) for t in range(TCH)]
        z1 = [ctx.enter_context(nc.psum_tensor(f"z1_{t}", [128, 2 * NT], F32)) for t in range(TCH)]
        z2 = [ctx.enter_context(nc.psum_tensor(f"z2_{t}", [128, 2 * NT], F32)) for t in range(TCH)]
        sem_x0 = ctx.enter_context(nc.semaphore("sem_x0"))
        sem_x0b = ctx.enter_context(nc.semaphore("sem_x0b"))
        sem_x1 = ctx.enter_context(nc.semaphore("sem_x1"))
        sem_x1b = ctx.enter_context(nc.semaphore("sem_x1b"))
        sem_w = ctx.enter_context(nc.semaphore("sem_w"))
        sem_b = ctx.enter_context(nc.semaphore("sem_b"))
        sem_o = ctx.enter_context(nc.semaphore("sem_o"))
        sem_mm = ctx.enter_context(nc.semaphore("sem_mm"))
        sem_act = ctx.enter_context(nc.semaphore("sem_act"))
        sem_v = ctx.enter_context(nc.semaphore("sem_v"))
        block = ctx.enter_context(nc.Block())

        # the W3 energy row reuses z2[t]'s first bank, partition 0 (its
        # matmuls run only after the Exps have drained z2[t])
        er = [z2[t][0:1, 0:NT] for t in range(TCH)]

        def w1s(d, h):
            return wt[:, (d * 2 + h) * 128 : (d * 2 + h + 1) * 128]

        def w2s(h, k):
            return wt[:, 768 + (h * 2 + k) * 128 : 768 + (h * 2 + k + 1) * 128]

        def w3s(k):
            return wt[:, 1280 + k : 1281 + k]

        @block.sync
        def _(sync):
            # host supplies x pre-laid-out as [128, t*(3*NT) + d*NT + s].
            # x is split across both HWDGE rings (SP + ACT) so the two rings
            # pull in parallel; d0+d1 of each chunk on SP, d2 on ACT.
            sync.dma_start(xt[:, 0 : 2 * NT], x_ext[:, 0 : 2 * NT]).then_inc(sem_x0, 16)
            sync.dma_start(xt[:, 3 * NT : 5 * NT], x_ext[:, 3 * NT : 5 * NT]).then_inc(sem_x1, 16)
            sync.dma_start(bias[:], bias_ext[:]).then_inc(sem_b, 16)
            sync.wait_ge(sem_v, 1)
            sync.dma_start(out_ext[:, 0:NT], out_sb[:, 0:NT]).then_inc(sem_o, 16)
            sync.wait_ge(sem_v, 2)
            sync.dma_start(out_ext[:, NT : 2 * NT], out_sb[:, NT : 2 * NT]).then_inc(sem_o, 16)
            sync.wait_ge(sem_o, 32)

        @block.scalar
        def _(scalar):
            scalar.dma_start(wt[:], wt_ext[:]).then_inc(sem_w, 16)
            scalar.dma_start(xt[:, 2 * NT : 3 * NT], x_ext[:, 2 * NT : 3 * NT]).then_inc(sem_x0b, 16)
            scalar.dma_start(xt[:, 5 * NT : 6 * NT], x_ext[:, 5 * NT : 6 * NT]).then_inc(sem_x1b, 16)
            # memzero lowers to an ACTIVATE, anchoring the ACT table load
            # before any cross-engine waits
            scalar.memzero(scratch[:])
            scalar.wait_ge(sem_b, 16)
            # PE sem_mm cumulative: z1(0)=1, z1(1)=2, z2(0)=3, z2(1)=4,
            # er(0)=5, er(1)=6.
            # sem_act: li 0..2 -> 3 incs each (exp,exp,ln); li 3 -> 4 incs
            # (exp,exp,ln half,ln half) so er(1) can start on the first half.
            for li, zz, tt, aa, bcol in (
                (0, z1, t1, a1, 0),
                (1, z1, t1, a1, 0),
                (2, z2, t2, a2, 2),
                (3, z2, t2, a2, 2),
            ):
                t = li % 2
                scalar.wait_ge(sem_mm, li + 1)
                for h in range(2):
                    scalar.activation(
                        tt[t][:, h * NT : (h + 1) * NT],
                        zz[t][:, h * NT : (h + 1) * NT],
                        EXP,
                        bias=bias[:, bcol + h : bcol + h + 1],
                    ).then_inc(sem_act, 1)
                scalar.wait_ge(sem_act, 3 * li + 2)  # ACT pipeline RAW: exp fully written
                if li < 3:
                    scalar.activation(aa[t][:], tt[t][:], LN, bias=1.0).then_inc(sem_act, 1)
                else:
                    for k in range(2):
                        scalar.activation(
                            aa[t][:, k * NT : (k + 1) * NT],
                            tt[t][:, k * NT : (k + 1) * NT],
                            LN,
                            bias=1.0,
                        ).then_inc(sem_act, 1)

        @block.tensor
        def _(tensor):
            def l1(t):
                for h in range(2):
                    for d in range(3):
                        mm = tensor.matmul(
                            z1[t][:, h * NT : (h + 1) * NT],
                            w1s(d, h),
                            xt[:, (t * 3 + d) * NT : (t * 3 + d + 1) * NT],
                            start=(d == 0),
                            stop=(d == 2),
                        )
                mm.then_inc(sem_mm, 1)

            def l2(t):
                for k in range(2):
                    for h in range(2):
                        mm = tensor.matmul(
                            z2[t][:, k * NT : (k + 1) * NT],
                            w2s(h, k),
                            a1[t][:, h * NT : (h + 1) * NT],
                            start=(h == 0),
                            stop=(h == 1),
                        )
                mm.then_inc(sem_mm, 1)

            def l3(t, act_waits):
                for k in range(2):
                    tensor.wait_ge(sem_act, act_waits[k])
                    mm = tensor.matmul(
                        er[t],
                        w3s(k),
                        a2[t][:, k * NT : (k + 1) * NT],
                        start=(k == 0),
                        stop=(k == 1),
                        skip_group_check=True,
                    )
                mm.then_inc(sem_mm, 1)

            tensor.wait_ge(sem_w, 16)
            tensor.wait_ge(sem_x0, 16)
            tensor.wait_ge(sem_x0b, 16)
            l1(0)  # -> 1
            tensor.wait_ge(sem_x1, 16)
            tensor.wait_ge(sem_x1b, 16)
            l1(1)  # -> 2
            tensor.wait_ge(sem_act, 3)
            l2(0)  # -> 3
            tensor.wait_ge(sem_act, 6)
            l2(1)  # -> 4
            l3(0, (9, 9))  # -> 5
            l3(1, (12, 13))  # -> 6

        @block.vector
        def _(vector):
            for t in range(TCH):
                vector.wait_ge(sem_mm, 5 + t)
                vector.tensor_scalar_add(
                    out_sb[:, t * NT : (t + 1) * NT], er[t], bias[0:1, 4:5]
                ).then_inc(sem_v, 1)

    nc.finalize()
    return nc


def kernel(representation, atomic_numbers, elements, W1, b1, W2, b2, W3, b3):
    global LAST_EXEC_NS
    rep = np.asarray(representation, dtype=np.float32)
    an = np.asarray(atomic_numbers).astype(np.int64)
    el = np.asarray(elements).astype(np.int64)
    W1 = np.asarray(W1, dtype=np.float32)
    b1 = np.asarray(b1, dtype=np.float32)
    W2 = np.asarray(W2, dtype=np.float32)
    b2 = np.asarray(b2, dtype=np.float32)
    W3 = np.asarray(W3, dtype=np.float32)
    b3 = np.asarray(b3, dtype=np.float32)

    Bsz, Nn, Dd = rep.shape
    flat = rep.reshape(-1, Dd)
    anf = an.reshape(-1)

    idxs = [np.nonzero(anf == el[e])[0] for e in range(E)]
    counts = [len(ix) for ix in idxs]

    # slots per core; expert capacity = 2*S (two cores per expert)
    S = 1024
    while max(counts) > 2 * S:
        S *= 2

    # fold the shifted-softplus -log(2) into downstream biases
    b2_eff = b2 - LOG2 * W2.sum(axis=1)  # [E, H2]
    b3_eff = b3 - LOG2 * W3.sum(axis=1)  # [E]

    if S not in _CACHE:
        _CACHE[S] = _build(S)
    nc = _CACHE[S]

    in_maps = []
    for c in range(N_CORES):
        e, half = divmod(c, 2)
        ix = idxs[e]
        lo = half * S
        hi = min(len(ix), lo + S)
        bf16 = ml_dtypes.bfloat16
        xs = np.zeros((S, Dd), np.float32)
        if hi > lo:
            xs[: hi - lo] = flat[ix[lo:hi]]
        wt = np.zeros((128, 1282), np.float32)
        wt[:, 0:768] = W1[e].reshape(3, 128, 2, 128).transpose(1, 0, 2, 3).reshape(128, 768)
        wt[:, 768:1280] = W2[e].reshape(2, 128, 2, 128).transpose(1, 0, 2, 3).reshape(128, 512)
        wt[:, 1280:1282] = W3[e].reshape(2, 128).T
        bias = np.zeros((128, 5), np.float32)
        bias[:, 0:2] = b1[e].reshape(2, 128).T
        bias[:, 2:4] = b2_eff[e].reshape(2, 128).T
        bias[0, 4] = b3_eff[e]
        in_maps.append(
            {
                "x": np.ascontiguousarray(
                    xs.T.reshape(3, 128, S // NT, NT).transpose(1, 2, 0, 3).reshape(128, 3 * S)
                ).astype(bf16),
                "wt": wt.astype(bf16),
                "bias": bias,
            }
        )

    kwargs = {}
    if PROFILE:
        kwargs = dict(trace=True, trace_cores=list(TRACE_CORES))
    res = run_bass_kernel_spmd(nc, in_maps, core_ids=list(range(N_CORES)), **kwargs)
    LAST_EXEC_NS = res.exec_time_ns

    energies = np.zeros(Bsz, np.float64)
    for c in range(N_CORES):
        e, half = divmod(c, 2)
        ix = idxs[e]
        lo = half * S
        hi = min(len(ix), lo + S)
        if hi <= lo:
            continue
        evals = np.asarray(res.results[c]["out"]).reshape(-1)[: hi - lo]
        np.add.at(energies, ix[lo:hi] // Nn, evals.astype(np.float64))
    return energies.astype(np.float32)
